# revision 37
# baseline (speedup 1.0000x reference)
"""GQA sparse-attention (sink + sliding window) kernel for 8 TRN2 NeuronCores.

Problem: nn_MultiHeadSelfAttentionModern (B=1, T=2048, D=2048, 32 q heads,
8 KV heads, d_head=64, WINDOW=2048, SINK=64, start_pos=2048, cache_len=2048).

Since S = cache_len + T = 4096 > WINDOW + SINK = 2112, the effective keys are
just kv_cache[:, :, :64] (the sink, used raw for both K and V) plus the 2048
new RoPE'd k (and raw new v).  Sharding: tensor-parallel by KV head — core i
owns KV head i and its 4 query heads, with Wq/Wk/Wv column-sharded and Wo
row-sharded; partial outputs are summed on the host (+ bo).

Design notes (cost model: PE matmul = out-free-cols x 0.42ns, ldweights free;
ACT exp ~1.04us per [*,1024] tile and exp runs ONLY on ACT):
  - bf16 datapath end to end (fp8 measured 3.2e-2 rel err vs the 2e-2 gate:
    infeasible)
  - FLIPPED ctx: out[t,d] = wT[s,t]^T @ v[s,d] per 128-token tile — wT is the
    stationary operand so the moving dim is d+1=65 instead of t, and the
    contraction is the full 128-wide s tile (the old layout moved t at
    M=65: 51% PE utilization).  Halves ctx PE time; denominators come out as
    a PSUM column via the ones-column in v_sb
  - per-token normalize: DVE reciprocal of the denominator column + one
    tensor_scalar_mul per (tile, head) — replaces the old bsel broadcast
    matmuls + DRAM round-trip entirely
  - normalized ctx O[t,d] is PE-transposed ([128,128] per tile-headpair,
    via a host-loaded bf16 identity: 1 cyc/row) into ctxT[d,t] for the
    unchanged output projection
  - v transposes (bf16 now) run as pend thunks inside phase 1 — flipped ctx
    consumes every s-tile within each token tile, so v_sb must be complete
    before the first ctx block
  - scores+exp stream unchanged ([s,t] layout, paced single-tile thunks);
    per-st feeds keep the in-order PE queue from coupling to the ACT pace
  - measured (cost-model timeline): see test.py; PE busy drops ~192->~164us
"""

import numpy as np

T = 2048
DMODEL = 2048
NKV = 8
GROUP = 4
DH = 64
SINK = 64
NST = 17  # s-tiles: 16 full 128-tiles of new tokens + 1 sink tile (64 rows)
SCALE = 0.125  # 1/sqrt(64)

_CACHE = {}


def _build_nc():
    import concourse.bass as bass
    import concourse.mybir as mybir
    import concourse.tile as tile
    from concourse import bacc

    f32 = mybir.dt.float32
    bf16 = mybir.dt.bfloat16

    nc = bacc.Bacc("TRN2", target_bir_lowering=False, debug=False, num_devices=NKV)

    xT = nc.declare_dram_parameter("xT", [DMODEL, T], bf16, isOutput=False).ap()
    wq01 = nc.declare_dram_parameter("wq01", [DMODEL, 2 * DH], bf16, isOutput=False).ap()
    wq23 = nc.declare_dram_parameter("wq23", [DMODEL, 2 * DH], bf16, isOutput=False).ap()
    wkv = nc.declare_dram_parameter("wkv", [DMODEL, 2 * DH], bf16, isOutput=False).ap()
    wo = nc.declare_dram_parameter("wo", [GROUP * DH, DMODEL], bf16, isOutput=False).ap()
    sink_kT = nc.declare_dram_parameter("sink_kT", [DH, SINK], bf16, isOutput=False).ap()
    sink_v = nc.declare_dram_parameter("sink_v", [128, DH], bf16, isOutput=False).ap()
    # cos and sin rope tables packed into one tensor: one DMA instead of
    # two (HWDGE descriptor generation is a serial global resource)
    cs_tbl = nc.declare_dram_parameter("cs_tbl", [128, 2 * T], bf16, isOutput=False).ap()
    # swap permutation: swp[p, i] = 1 iff p = (i+32 mod 64) within i's 64-block
    swp = nc.declare_dram_parameter("swp", [128, 128], bf16, isOutput=False).ap()
    # bf16 identity for PE transposes (1 cycle/row vs 2 for f32)
    id128 = nc.declare_dram_parameter("id128", [128, 128], bf16, isOutput=False).ap()
    out = nc.declare_dram_parameter("out", [T, DMODEL], bf16, isOutput=True).ap()

    with (
        tile.TileContext(nc) as tc,
        tc.tile_pool(name="persist", bufs=1) as persist,
        # scores psum (4 banks double buffered): the exp stream must never
        # wait on projection psum
        tc.tile_pool(name="psA", bufs=2, space="PSUM") as psA,
        tc.tile_pool(name="pm", bufs=1) as pm,
    ):
        # q01/q23: heads (0,1) and (2,3) on partition halves; odd q heads are
        # extracted to base-partition-0 tiles qx1/qx3 post-rope.
        q01 = persist.tile([128, T], bf16, tag="q01")
        q23 = persist.tile([128, T], bf16, tag="q23")
        qx1 = persist.tile([DH, T], bf16, tag="qx1")
        qx3 = persist.tile([DH, T], bf16, tag="qx3")
        kk = persist.tile([DH, T], bf16, tag="kk")
        vTt = persist.tile([DH, T], bf16, tag="vTt")
        v_sb = persist.tile([128, NST, DH + 1], bf16, tag="v_sb")
        ctxT = [persist.tile([128, T], bf16, tag=f"ctxT{j}", name=f"ctxT{j}") for j in range(2)]
        # normalized ctx in [token, head-dim] layout: O_sb[:, ti, g*64:(g+1)*64]
        O_sb = persist.tile([128, 16, GROUP * DH], bf16, tag="O_sb")
        sink_kT_sb = persist.tile([DH, SINK], bf16, tag="sink_kT")
        # merged sink weights: one tile per block PAIR — head A's sink exp
        # rows live at partitions 0:64, head B's at 64:128 (one exp instead
        # of two; the base-64 ctx matmul reads the duplicated sink v rows)
        wTs = [
            persist.tile([128, 1024], bf16, tag=f"wTs{p}", name=f"wTs{p}")
            for p in range(4)
        ]
        # block (0,3)'s first 4 s-tiles: a small dedicated tile lets head 3
        # join the early exp stream without a wT rotation conflict (its
        # rotating buffer is still owned by block (0,0) during phase 1)
        wT03e = persist.tile([128, 4, 1024], bf16, tag="wT03e")
        swp_sb = persist.tile([128, 128], bf16, tag="swp")
        id128_sb = persist.tile([128, 128], bf16, tag="id128_sb")

        xTr = xT.rearrange("(k p) t -> p k t", p=128)
        wkvr = wkv.rearrange("(k p) m -> p k m", p=128)
        wq01r = wq01.rearrange("(k p) m -> p k m", p=128)
        wq23r = wq23.rearrange("(k p) m -> p k m", p=128)

        # wT buffers rotate out of pm, triple buffered (sink weights live
        # in the per-pair wTs tiles instead)
        def new_wT():
            return pm.tile([128, 16, 1024], bf16, tag="wT", bufs=3, name="wTm")

        def emit_tile(qt, wT, c0, st):
            """One s-tile of scores (PE) + its exp (ACT).

            The PE queue is in-order and scores psum is double buffered, so
            bursts of more than ~2 of these couple the PE to the ACT exp
            pace; callers must interleave them with other PE work.
            """
            sps = psA.tile([128, 1024], f32, tag="sps", name="sps")
            lhsT = kk[:, st * 128 : (st + 1) * 128]
            for u in range(2):
                nc.tensor.matmul(
                    sps[:, u * 512 : (u + 1) * 512],
                    lhsT=lhsT,
                    rhs=qt[:, c0 + u * 512 : c0 + (u + 1) * 512],
                    start=True,
                    stop=True,
                )
            nc.scalar.activation(
                out=wT[:, st, :],
                in_=sps,
                func=mybir.ActivationFunctionType.Exp,
                scale=SCALE,
            )

        def emit_half(qt, wT, st, u, c0=0):
            """512-wide emission: half the block width per exp.  Used to
            start the ACT exp stream as soon as chunk 0 is roped, and to
            split the last block so its ctx/output chain overlaps the final
            exps (costs one extra exp's fixed overhead per tile)."""
            sps = psA.tile([128, 512], f32, tag="sps", name="sps")
            lhsT = kk[:, st * 128 : (st + 1) * 128]
            nc.tensor.matmul(
                sps,
                lhsT=lhsT,
                rhs=qt[:, c0 + u * 512 : c0 + (u + 1) * 512],
                start=True,
                stop=True,
            )
            nc.scalar.activation(
                out=wT[:, st, u * 512 : (u + 1) * 512],
                in_=sps,
                func=mybir.ActivationFunctionType.Exp,
                scale=SCALE,
            )

        def emit_sink(pair, qA, qB, c0=0, u=None):
            """Merged sink scores for a block pair: head A's rows at
            partitions 0:64, head B's at 64:128, one exp for both."""
            if u is None:
                w, sl = 1024, slice(0, 1024)
            else:
                w, sl = 512, slice(u * 512, (u + 1) * 512)
            sps = psA.tile([128, w], f32, tag="sps", name="sps")
            for hb, qt in ((0, qA), (1, qB)):
                for uu in range(w // 512):
                    t0 = c0 + (sl.start if u is not None else uu * 512)
                    nc.tensor.matmul(
                        sps[hb * 64 : hb * 64 + 64, uu * 512 : uu * 512 + 512],
                        lhsT=sink_kT_sb,
                        rhs=qt[:, t0 : t0 + 512],
                        start=True,
                        stop=True,
                    )
            nc.scalar.activation(
                out=wTs[pair][:, sl],
                in_=sps,
                func=mybir.ActivationFunctionType.Exp,
                scale=SCALE,
            )

        def transpose_v(st):
            # v^T -> v_sb through a scores-psum slot (bf16 transpose via the
            # bf16 identity: 1 cyc/row); no extra PSUM banks
            sps = psA.tile([128, 1024], bf16, tag="sps", name="sps")
            nc.tensor.transpose(
                sps[:, 0:DH], vTt[:, st * 128 : (st + 1) * 128],
                id128_sb[0:DH, 0:DH],
            )
            nc.vector.tensor_copy(out=v_sb[:, st, 0:DH], in_=sps[:, 0:DH])

        pend = []  # queued emission thunks, paced via pump()

        def pump(n):
            for _ in range(min(n, len(pend))):
                pend.pop(0)()

        with (
            tc.tile_pool(name="psB", bufs=1, space="PSUM") as psB,
            tc.tile_pool(name="pw", bufs=1) as pw,
            tc.tile_pool(name="px", bufs=2) as px,
        ):
            # ---- phase 1: projections + per-chunk rope ----
            # first weight piece alone so the k=0 matmuls start at ~1.5us
            # lo/hi halves as separate tiles: reader-after-writer ordering
            # is per tile, so k<8 matmuls issued after the hi-half DMAs must
            # not inherit a dependency on them
            wq01_lo = pw.tile([128, 8, 2 * DH], bf16, tag="wq01_lo")
            wq01_hi = pw.tile([128, 8, 2 * DH], bf16, tag="wq01_hi")
            wq23_lo = pw.tile([128, 8, 2 * DH], bf16, tag="wq23_lo")
            wq23_hi = pw.tile([128, 8, 2 * DH], bf16, tag="wq23_hi")
            wkv_lo = pw.tile([128, 8, 2 * DH], bf16, tag="wkv_lo")
            wkv_hi = pw.tile([128, 8, 2 * DH], bf16, tag="wkv_hi")

            def wq01s(k):
                return wq01_lo[:, k] if k < 8 else wq01_hi[:, k - 8]

            def wq23s(k):
                return wq23_lo[:, k] if k < 8 else wq23_hi[:, k - 8]

            def wkvs(k):
                return wkv_lo[:, k] if k < 8 else wkv_hi[:, k - 8]


            sinkv_st = pw.tile([128, DH], bf16, tag="sinkv_st")

            def rope_chunk(tgt, cs, pp, cs_c):
                """tgt[0:pp, cs] <- tgt*C + swap32(tgt)*S on token slice cs;
                cs_c holds this chunk's 512 cos cols then 512 sin cols."""
                n = cs.stop - cs.start
                aux = psB.tile([128, 512], f32, tag="aux", name="aux")
                nc.tensor.matmul(
                    aux[0:pp, 0:n],
                    lhsT=swp_sb[0:pp, 0:pp],
                    rhs=tgt[0:pp, cs],
                    start=True,
                    stop=True,
                )
                swt = px.tile([128, 512], bf16, tag="sw")
                nc.vector.tensor_mul(swt[0:pp, 0:n], aux[0:pp, 0:n], cs_c[0:pp, 512 : 512 + n])
                nc.vector.tensor_mul(tgt[0:pp, cs], tgt[0:pp, cs], cs_c[0:pp, 0:n])
                nc.vector.tensor_add(tgt[0:pp, cs], tgt[0:pp, cs], swt[0:pp, 0:n])

            wT_early = {}
            qsrc = [q01[0:DH, :], qx1, q23[0:DH, :], qx3]

            # PE p-state ramp: matmuls run 2-4x slow until ~3us of
            # continuous PE activity.  Warm the ramp on a zero tile while
            # the first x/weight DMAs are still in flight, so the real
            # projection stream starts at full speed.
            warm = px.tile([128, 128], bf16, tag="warm", bufs=1)
            nc.vector.memset(warm, 0.0)
            wps = psB.tile([128, 512], f32, tag="aux", name="wps")
            for _ in range(4):
                nc.tensor.matmul(
                    wps[:, 0:128], lhsT=warm, rhs=warm, start=True, stop=True
                )

            for c in range(4):  # token chunks of 512
                cs_c = pw.tile([128, 1024], bf16, tag="cs", bufs=2, name="cs_c")
                q01ps = psB.tile([128, 512], f32, tag="q01ps")
                q23ps = psB.tile([128, 512], f32, tag="q23ps")
                kvps = psB.tile([128, 512], f32, tag="kvps")
                cs = slice(c * 512, (c + 1) * 512)
                # kv + q01 matmuls stream per x quad; q23's are deferred past
                # the kk/q01 ropes (they gate the early exp stream), so the
                # first exp fires after ~70 PE matmuls instead of ~105
                xts = []
                for quad in range(4):
                    # DMA queues: transfers serialize per HWDGE queue, so the
                    # startup x stream alternates the SP and ACT queues (ACT
                    # is idle until its first exp ~15us) and every constant
                    # rides the DVE queue — chunk 0 then lands by ~5us
                    # instead of ~14us and the exp stream starts ~8us sooner.
                    if c == 0 and quad == 0:
                        # first 2 x pieces + first weight halves alone so the
                        # first projection matmul starts at ~2.5us.  The DMA
                        # transfer stage and HWDGE are single global
                        # resources, so issue ORDER is everything: hi weights
                        # after quad 1 (needed at k=8), tables after quad 2
                        # (needed by the ropes).
                        xa = px.tile([128, 1, 512], bf16, tag="xt0", bufs=1)
                        nc.sync.dma_start(out=xa, in_=xTr[:, 0:1, cs])
                        nc.sync.dma_start(out=wkv_lo[:, 0:2], in_=wkvr[:, 0:2])
                        nc.sync.dma_start(out=wq01_lo[:, 0:2], in_=wq01r[:, 0:2])
                        xb = px.tile([128, 3, 512], bf16, tag="xt1", bufs=1)
                        nc.sync.dma_start(out=xb, in_=xTr[:, 1:4, cs])
                        nc.sync.dma_start(out=wkv_lo[:, 2:4], in_=wkvr[:, 2:4])
                        nc.sync.dma_start(out=wq01_lo[:, 2:4], in_=wq01r[:, 2:4])
                        nc.sync.dma_start(out=wkv_lo[:, 4:8], in_=wkvr[:, 4:8])
                        nc.sync.dma_start(out=wq01_lo[:, 4:8], in_=wq01r[:, 4:8])
                        parts = [(xa, 0, 1), (xb, 1, 3)]
                    else:
                        xt = px.tile([128, 4, 512], bf16, tag="xt", bufs=7)
                        nc.sync.dma_start(
                            out=xt,
                            in_=xTr[:, quad * 4 : (quad + 1) * 4, cs],
                        )
                        parts = [(xt, quad * 4, 4)]
                        if c == 0 and quad == 1:
                            # hi weight halves: consumed by this chunk's k=8
                            nc.sync.dma_start(out=wkv_hi, in_=wkvr[:, 8:16])
                            nc.sync.dma_start(out=wq01_hi, in_=wq01r[:, 8:16])
                        elif c == 0 and quad == 3:
                            # rope tables + sink keys AFTER the last x quad,
                            # and only chunk 0's 512-col cos/sin slices: the
                            # ropes are the exp-stream gate and only ever
                            # need the current chunk's columns
                            nc.sync.dma_start(
                                out=cs_c[:, 0:512], in_=cs_tbl[:, 0:512]
                            )
                            nc.sync.dma_start(
                                out=cs_c[:, 512:1024], in_=cs_tbl[:, T : T + 512]
                            )
                            nc.sync.dma_start(out=swp_sb, in_=swp)
                            nc.sync.dma_start(out=sink_kT_sb, in_=sink_kT)
                            # q23 weights leave the critical front: their
                            # matmuls only run after the early emits
                            nc.sync.dma_start(out=wq23_lo, in_=wq23r[:, 0:8])
                            nc.sync.dma_start(out=wq23_hi, in_=wq23r[:, 8:16])
                        elif c >= 1 and quad == 0:
                            # this chunk's cos/sin slices: land well before
                            # its ropes
                            nc.sync.dma_start(
                                out=cs_c[:, 0:512],
                                in_=cs_tbl[:, c * 512 : (c + 1) * 512],
                            )
                            nc.sync.dma_start(
                                out=cs_c[:, 512:1024],
                                in_=cs_tbl[:, T + c * 512 : T + (c + 1) * 512],
                            )
                        if c == 1 and quad == 0:
                            # consumed by the v transposes / phase 2; SWDGE
                            # keeps the HWDGE queue free
                            nc.gpsimd.dma_start(out=sinkv_st, in_=sink_v)
                            nc.gpsimd.dma_start(out=id128_sb, in_=id128)
                    xts.extend(parts)
                    for xtile, k0, kn in parts:
                        for k4 in range(kn):
                            k = k0 + k4
                            nc.tensor.matmul(
                                kvps, lhsT=wkvs(k),
                                rhs=xtile[:, k4, :], start=(k == 0), stop=(k == 15),
                            )
                            nc.tensor.matmul(
                                q01ps, lhsT=wq01s(k), rhs=xtile[:, k4, :],
                                start=(k == 0), stop=(k == 15),
                            )
                        pump(2)
                nc.vector.tensor_copy(out=kk[:, cs], in_=kvps[0:DH, :])
                nc.vector.tensor_copy(out=q01[:, cs], in_=q01ps)
                nc.vector.tensor_copy(out=vTt[:, cs], in_=kvps[DH:128, :])
                if c > 0:
                    pump(2)  # queued scores read already-roped chunks: free
                    # PE work while the kk/q01 copies land
                # rope (k first: it gates all heads' scores); pumped tiles
                # read already-roped chunks, covering the copy/swap latency
                rope_chunk(kk, cs, DH, cs_c)
                pump(1)
                rope_chunk(q01, cs, 128, cs_c)
                pump(1)
                # odd head to a base-partition-0 tile (post-rope, per chunk);
                # SWDGE: keeps the contended HWDGE free for x/weight loads
                nc.gpsimd.dma_start(out=qx1[:, cs], in_=q01[DH:128, cs])
                # early exp: queue scores+exp for the chunk-0 blocks of heads
                # 0/1 as soon as the needed k/q chunks are roped; pump() paces
                # them between projection quads so the ACT engine (the ~141us
                # serial backbone) starts early without ever blocking the
                # in-order PE queue on the exp stream
                if c == 0:
                    # earliest possible exp stream: 512-wide score halves for
                    # the sink + chunk-0 keys x chunk-0 queries of heads 0/1,
                    # pumped by the remaining q23 parts below
                    wT_early[0] = new_wT()
                    wT_early[1] = new_wT()
                    pend.extend(
                        (lambda st=st: emit_half(qsrc[0], wT_early[0], st, 0))
                        for st in range(0, 4)
                    )
                    pend.append(lambda: emit_sink(0, qsrc[0], qsrc[1], u=0))
                    pend.extend(
                        (lambda st=st: emit_half(qsrc[1], wT_early[1], st, 0))
                        for st in range(0, 4)
                    )
                    # fire the early emits BEFORE the q23 matmuls: q23 waits
                    # on its late-loaded weights, and the PE has nothing else
                    # this early — a paced cram here is free
                    pump(5)
                    for xtile, k0, kn in xts:
                        for k4 in range(kn):
                            k = k0 + k4
                            nc.tensor.matmul(
                                q23ps, lhsT=wq23s(k),
                                rhs=xtile[:, k4, :],
                                start=(k == 0), stop=(k == 15),
                            )
                        pump(2)
                elif c == 1:
                    pend.append(lambda: emit_sink(0, qsrc[0], qsrc[1], u=1))
                    for g in range(2):
                        pend.extend(
                            (lambda g=g, st=st: emit_half(qsrc[g], wT_early[g], st, 1))
                            for st in range(0, 4)
                        )
                        pend.extend(
                            (lambda g=g, st=st: emit_tile(qsrc[g], wT_early[g], 0, st))
                            for st in range(4, 8)
                        )
                elif c >= 2:
                    # g=2 (block (0,2), wT02) joins here: its q23 chunks 0/1
                    # were roped in earlier chunks, so full-width is fine
                    for g in range(3):
                        pend.extend(
                            (lambda g=g, st=st: emit_tile(qsrc[g], wT_early[g], 0, st))
                            for st in range(4 * c, 4 * c + 4)
                        )
                # deferred q23 matmuls + its rope
                if c > 0:
                    for xtile, k0, kn in xts:
                        for k4 in range(kn):
                            k = k0 + k4
                            nc.tensor.matmul(
                                q23ps, lhsT=wq23s(k), rhs=xtile[:, k4, :],
                                start=(k == 0), stop=(k == 15),
                            )
                        pump(3)
                nc.vector.tensor_copy(out=q23[:, cs], in_=q23ps)
                rope_chunk(q23, cs, 128, cs_c)
                nc.gpsimd.dma_start(out=qx3[:, cs], in_=q23[DH:128, cs])
                # block (0,2)'s early tiles queue only after q23's rope: its
                # scores read the q23 chunk roped just above
                if c == 0:
                    wT_early[2] = new_wT()
                    pend.append(lambda: emit_sink(1, qsrc[2], qsrc[3], u=0))
                    for st in range(0, 4):
                        pend.append(
                            lambda st=st: emit_half(qsrc[2], wT_early[2], st, 0)
                        )
                        pend.append(
                            lambda st=st: emit_half(qsrc[3], wT03e, st, 0)
                        )
                elif c == 1:
                    pend.append(lambda: emit_sink(1, qsrc[2], qsrc[3], u=1))
                    for st in range(0, 4):
                        pend.append(
                            lambda st=st: emit_half(qsrc[2], wT_early[2], st, 1)
                        )
                        pend.append(
                            lambda st=st: emit_half(qsrc[3], wT03e, st, 1)
                        )
                    pend.extend(
                        (lambda st=st: emit_tile(qsrc[2], wT_early[2], 0, st))
                        for st in range(4, 8)
                    )
                # v transposes queue AFTER the chunk's emits: they carry no
                # exp work, so they must not head the FIFO ahead of scores
                # (v_sb only has to be complete before the first ctx block).
                # Chunk 3's go FIRST so the deferred tail below holds only
                # emits that ctx(0,0) may safely pace.
                if c == 1:
                    for st in range(0, 8):
                        pend.append(lambda st=st: transpose_v(st))
                elif c == 2:
                    for st in range(8, 12):
                        pend.append(lambda st=st: transpose_v(st))
                elif c == 3:
                    for st in range(12, 16):
                        pend.append(lambda st=st: transpose_v(st))
                    # c=1 block pairs' merged sinks: all q is roped now
                    pend.append(lambda: emit_sink(2, qsrc[0], qsrc[1], c0=1024))
                    pend.append(lambda: emit_sink(3, qsrc[2], qsrc[3], c0=1024))
            pump(len(pend))  # drain any leftovers (ACT is behind PE here)
            # phase-2-only constants, placed after the chunk loop so their
            # consumers never head-block the in-order DVE/Pool queues while
            # the projection copies are pending
            nc.vector.tensor_copy(out=v_sb[:, NST - 1, 0:DH], in_=sinkv_st)
            nc.vector.memset(v_sb[:, :, DH : DH + 1], 1.0)

        # ---- phase 2+3: flipped ctx + transposes, then per-512 output ----
        with (
            tc.tile_pool(name="psOC", bufs=2, space="PSUM") as psOC,
            tc.tile_pool(name="psD", bufs=2, space="PSUM") as psD,
            tc.tile_pool(name="pLate", bufs=1) as pLate,
        ):
            wo_sb = pLate.tile([128, 2, DMODEL], bf16, tag="wo_sb")
            nc.sync.dma_start(out=wo_sb, in_=wo.rearrange("(a p) n -> p a n", p=128))

            # yps and transpose psums share one tag (bank-granular slots):
            # psA's 4 banks + psOC's 2 + these 2 fill PSUM exactly
            def scratch_ps():
                return psD.tile([128, 512], f32, tag="ps", name="ps")

            # --- rate-paced emission: the in-order PE queue means a queued
            # emit blocks ALL later PE work until its scores psum frees (the
            # ACT exp two back).  Feeds must therefore be spread at no more
            # than ~1 per exp-duration of interleaved PE work, or real work
            # gets pushed past the end of the exp stream.  slot(ns) is called
            # at every interleave point with the PE-ns just appended and pops
            # pending emits at their own pace.
            pend2 = []  # (thunk, pace_ns, group)
            pacc = [0.0]

            def slot(ns):
                pacc[0] += ns
                pops = 0
                while pend2 and pacc[0] >= pend2[0][1] and pops < 2:
                    _, pace, _ = pend2[0]
                    pacc[0] -= pace
                    pend2.pop(0)[0]()
                    pops += 1

            def queue_block(thunks, pace, group=""):
                if not pend2:
                    pacc[0] = min(pacc[0], 1000.0)
                pend2.extend((t, pace, group) for t in thunks)

            def drain_upto(group):
                """Force-run queued emits from the front until none of the
                given group remain (safety net under the slot() pacing —
                every ctx must have its block's exps fully issued)."""
                while any(g == group for _, _, g in pend2):
                    pend2.pop(0)[0]()

            def full_tiles(c, g, wT):
                return [
                    (lambda st=st: emit_tile(qsrc[g], wT, c * 1024, st))
                    for st in range(16)
                ]

            def ctx_block(c, g, wT, tts=range(8), wt_e4=None):
                """Flipped ctx for one block: per 128-token tile, the merged
                sink tile first (head parity picks the partition half), then
                the 16 key tiles, all with wT as the stationary operand;
                normalize into O_sb via the denominator column."""
                hb = g % 2
                pair = (c << 1) | (g >> 1)
                for tt in tts:
                    oc = psOC.tile([128, DH + 1], f32, tag="oc", name="oc")
                    cols = slice(tt * 128, (tt + 1) * 128)
                    slot(232)
                    nc.tensor.matmul(
                        oc,
                        lhsT=wTs[pair][hb * 64 : hb * 64 + 64, cols],
                        rhs=v_sb[hb * 64 : hb * 64 + 64, NST - 1, :],
                        start=True,
                        stop=False,
                    )
                    for i, st in enumerate(range(16)):
                        if i == 8:
                            slot(232)
                        src_wT = wt_e4 if (wt_e4 is not None and st < 4) else wT
                        nc.tensor.matmul(
                            oc,
                            lhsT=src_wT[:, st, cols],
                            rhs=v_sb[:, st, :],
                            start=False,
                            stop=(i == 15),
                        )
                    rec = pm.tile([128, 1], f32, tag="rec", bufs=2, name="rec")
                    nc.vector.reciprocal(rec, oc[:, DH : DH + 1])
                    nc.vector.tensor_scalar_mul(
                        O_sb[:, c * 8 + tt, g * DH : (g + 1) * DH],
                        oc[:, 0:DH], rec,
                    )

            def tp_burst(c, j, tts=range(8)):
                """O_sb[t, headpair j] -> ctxT[j][d, t] for the tiles of
                token half c (PE transpose + DVE drain per tile)."""
                for tt in tts:
                    slot(181)
                    tp = psD.tile([128, 128], bf16, tag="ps", name="tp")
                    nc.tensor.transpose(
                        tp, O_sb[:, c * 8 + tt, j * 128 : (j + 1) * 128], id128_sb
                    )
                    ti = c * 8 + tt
                    nc.vector.tensor_copy(
                        out=ctxT[j][:, ti * 128 : (ti + 1) * 128], in_=tp
                    )

            def ny_block(c, u, mode):
                """Output projection for 512 tokens.  mode: "steady" (DVE
                copies, psD yps), "mid" (exp stream still running: DVE-only
                copies, yps alternating psD/psOC — psOC's ctx accumulators
                are done by then — to break the two-buffer ping-pong without
                touching psA or the ACT queue), "tail" (exps over: copies
                alternate DVE/ACT, yps alternate psD/psA)."""
                t0 = c * 1024 + u * 512
                for tt4 in range(4):  # output projection per 128 tokens
                    tt = t0 // 128 + tt4
                    y_sb = pLate.tile([128, DMODEL], bf16, tag="y_sb", bufs=3, name="y_sb")
                    for nck in range(4):
                        slot(427)
                        if mode == "tail" and nck % 2 == 1:
                            yps = psA.tile([128, 512], f32, tag="sps", name="sps")
                        elif mode == "mid" and nck % 2 == 1:
                            yps = psOC.tile([128, 512], f32, tag="oc", name="oc")
                        else:
                            yps = scratch_ps()
                        for j in range(2):
                            nc.tensor.matmul(
                                yps,
                                lhsT=ctxT[j][:, tt * 128 : (tt + 1) * 128],
                                rhs=wo_sb[:, j, nck * 512 : (nck + 1) * 512],
                                start=(j == 0),
                                stop=(j == 1),
                            )
                        ysl = slice(nck * 512, (nck + 1) * 512)
                        if mode == "tail" and nck % 2 == 1:
                            # ACT is idle once its exp stream ends; share the
                            # tail copies between DVE and ACT
                            nc.scalar.copy(out=y_sb[:, ysl], in_=yps)
                        else:
                            nc.vector.tensor_copy(out=y_sb[:, ysl], in_=yps)
                        if mode == "tail" and tt4 == 3 and nck == 1:
                            # very last tile: DMA the first half early so the
                            # final transfer overlaps the remaining copies
                            nc.sync.dma_start(
                                out=out[tt * 128 : (tt + 1) * 128, 0:1024],
                                in_=y_sb[:, 0:1024],
                            )
                    if mode == "tail" and tt4 == 3:
                        nc.sync.dma_start(
                            out=out[tt * 128 : (tt + 1) * 128, 1024:2048],
                            in_=y_sb[:, 1024:2048],
                        )
                    else:
                        nc.sync.dma_start(out=out[tt * 128 : (tt + 1) * 128, :], in_=y_sb)

            # software pipeline.  wT rotation (bufs=3, allocations e0,e1,e2
            # in phase 1 then w03,w10,w11,w12,w13): each block's emission can
            # only be queued after the ctx that reads the buffer it reuses.
            # The last block (1,3) is emitted as 512-wide halves so its
            # ctx/transpose/output chain for the first half overlaps the
            # second half's exps.
            ctx_block(0, 0, wT_early[0])
            wT03 = new_wT()
            queue_block(full_tiles(0, 3, wT03)[4:], 1000, "03")
            ctx_block(0, 1, wT_early[1])
            wT10 = new_wT()
            queue_block(full_tiles(1, 0, wT10), 1000, "10")
            tp_burst(0, 0)
            ctx_block(0, 2, wT_early[2])
            wT11 = new_wT()
            queue_block(full_tiles(1, 1, wT11), 1000, "11")
            drain_upto("03")
            ctx_block(0, 3, wT03, wt_e4=wT03e)
            wT12 = new_wT()
            queue_block(full_tiles(1, 2, wT12), 1000, "12")
            tp_burst(0, 1)
            ny_block(0, 0, "steady")
            drain_upto("10")
            ctx_block(1, 0, wT10)
            wT13 = new_wT()
            queue_block(
                [
                    (lambda st=st: emit_half(qsrc[3], wT13, st, 0, c0=1024))
                    for st in range(16)
                ],
                620, "a",
            )
            queue_block(
                [
                    (lambda st=st: emit_half(qsrc[3], wT13, st, 1, c0=1024))
                    for st in range(16)
                ],
                620, "b",
            )
            ny_block(0, 1, "steady")
            drain_upto("11")
            ctx_block(1, 1, wT11)
            tp_burst(1, 0)
            drain_upto("12")
            ctx_block(1, 2, wT12)
            drain_upto("a")
            ctx_block(1, 3, wT13, tts=range(0, 4))
            tp_burst(1, 1, tts=range(0, 4))
            ny_block(1, 0, "mid")
            drain_upto("b")
            ctx_block(1, 3, wT13, tts=range(4, 8))
            tp_burst(1, 1, tts=range(4, 8))
            ny_block(1, 1, "tail")

    nc.compile()
    return nc


def _host_inputs(x, kv_cache, Wq, Wk, Wv, Wo, start_pos):
    """Build the 8 per-core input dicts."""
    from ml_dtypes import bfloat16

    f32 = np.float32
    xT = np.ascontiguousarray(np.asarray(x, f32)[0].T.astype(bfloat16))

    inv_freq = (1.0 / (10000.0 ** (np.arange(0, DH, 2, dtype=f32) / DH))).astype(f32)
    pos = np.arange(start_pos, start_pos + T, dtype=f32)
    ang = pos[:, None] * inv_freq[None, :]
    cosT = np.cos(ang).T.astype(f32)  # (32, T)
    sinT = np.sin(ang).T.astype(f32)
    cosb = np.concatenate([cosT] * 4, axis=0)
    sinb = np.concatenate([-sinT, sinT, -sinT, sinT], axis=0)
    cs_tbl = np.ascontiguousarray(np.concatenate([cosb, sinb], axis=1)).astype(bfloat16)

    # 32-row swap within each 64-block: swp[p, i] = 1 iff p = swap(i)
    swp = np.zeros((128, 128), dtype=bfloat16)
    for i in range(128):
        blk = (i // 64) * 64
        swp[blk + ((i - blk) + 32) % 64, i] = 1
    id128 = np.eye(128, dtype=bfloat16)

    Wq = np.asarray(Wq, f32)
    Wk = np.asarray(Wk, f32)
    Wv = np.asarray(Wv, f32)
    Wo = np.asarray(Wo, f32)
    kv_cache = np.asarray(kv_cache, f32)

    in_maps = []
    for i in range(NKV):
        sink = kv_cache[0, i, :SINK, :]
        sink_kT = np.ascontiguousarray(sink.T).astype(bfloat16)
        in_maps.append(
            {
                "xT": xT,
                "wq01": np.ascontiguousarray(
                    Wq[:, i * GROUP * DH : i * GROUP * DH + 2 * DH]
                ).astype(bfloat16),
                "wq23": np.ascontiguousarray(
                    Wq[:, i * GROUP * DH + 2 * DH : (i + 1) * GROUP * DH]
                ).astype(bfloat16),
                "wkv": np.ascontiguousarray(
                    np.concatenate(
                        [Wk[:, i * DH : (i + 1) * DH], Wv[:, i * DH : (i + 1) * DH]],
                        axis=1,
                    )
                ).astype(bfloat16),
                "wo": np.ascontiguousarray(
                    Wo[i * GROUP * DH : (i + 1) * GROUP * DH, :]
                ).astype(bfloat16),
                "sink_kT": sink_kT,
                "sink_v": np.ascontiguousarray(
                    np.concatenate([sink, sink], axis=0)
                ).astype(bfloat16),
                "cs_tbl": cs_tbl,
                "swp": swp,
                "id128": id128,
            }
        )
    return in_maps


def run(inputs, trace=False, trace_kwargs=None):
    """Run the 8-core kernel; returns (y, BassKernelResults)."""
    from concourse.bass_utils import run_bass_kernel_spmd

    if "nc" not in _CACHE:
        _CACHE["nc"] = _build_nc()
    nc = _CACHE["nc"]

    start_pos = int(np.asarray(inputs["start_pos"]))
    in_maps = _host_inputs(
        inputs["x"], inputs["kv_cache"], inputs["Wq"], inputs["Wk"], inputs["Wv"],
        inputs["Wo"], start_pos,
    )
    kwargs = {}
    if trace:
        kwargs["trace"] = True
        if trace_kwargs:
            kwargs["trace_kwargs"] = trace_kwargs
    res = run_bass_kernel_spmd(nc, in_maps, core_ids=list(range(NKV)), **kwargs)

    y = res.results[0]["out"].astype(np.float64)
    for i in range(1, NKV):
        y += res.results[i]["out"]
    y = (y + np.asarray(inputs["bo"], np.float64)[None, :]).astype(np.float32)
    return y[None], res


def kernel(**inputs):
    y, _ = run(inputs)
    return y


# revision 40
# speedup vs baseline: 1.0891x; 1.0891x over previous
"""GQA sparse-attention (sink + sliding window) kernel for 8 TRN2 NeuronCores.

Problem: nn_MultiHeadSelfAttentionModern (B=1, T=2048, D=2048, 32 q heads,
8 KV heads, d_head=64, WINDOW=2048, SINK=64, start_pos=2048, cache_len=2048).

Since S = cache_len + T = 4096 > WINDOW + SINK = 2112, the effective keys are
just kv_cache[:, :, :64] (the sink, used raw for both K and V) plus the 2048
new RoPE'd k (and raw new v).  Sharding: tensor-parallel by KV head - core i
owns KV head i and its 4 query heads, with Wq/Wk/Wv column-sharded and Wo
row-sharded; partial outputs are summed on the host (+ bo).

Design (cost model: PE matmul = out-free-cols x 0.42ns, ldweights free; exp
runs ONLY on ACT at ~0.83ns/col + ~190ns/instr; DMA transfer + HWDGE are
single global serial resources):
  - bf16 datapath (fp8 measured 3.2e-2 rel err vs the 2e-2 gate: infeasible)
  - FLIPPED ctx: out[t,d+1] = wT[s,t]^T @ v[s,d+1] per 128-token tile - wT
    stationary, so the moving dim is 65 instead of 1024 and the contraction
    uses the full 128-wide s tile (the old layout ran at 51% PE util).
    Halves ctx PE time (192.5us PE busy -> 161); denominators come out as a
    psum column via the ones-column in v_sb
  - per-token normalize: DVE reciprocal + one tensor_scalar_mul per
    (tile, head); normalized O[t,d] is PE-transposed (bf16 identity, 1
    cyc/row) into ctxT[d,t] for the unchanged output projection
  - merged sink exps: block PAIRS share one sink tile (head A rows 0:64,
    head B 64:128, duplicated sink-v rows; ctx reads via base-64 matmuls) -
    one exp per pair instead of per block
  - exp stream starts ~22us: PE p-state warmup matmuls at ~1us, wq split
    into q01/q23 halves so q23 weights leave the critical front-DMA window,
    per-chunk cos/sin slices, and 512-wide early emission of every head's
    chunk-0 scores (block (0,3) via a small dedicated wT03e tile that dodges
    the wT rotation)
  - all later emission is rate-paced (pend2/slot): the in-order PE queue
    means a queued emit blocks all later PE work until its scores psum frees
    (the exp two back), so feeds are popped at ~the exp rate against PE
    work appended; paces tuned empirically (550-700ns)
  - last block (1,3) emits as 512-wide halves: its first-half ctx/transpose/
    output chain overlaps the second half's exps; tail ny alternates yps out
    of psA (free after the stream) and copies between DVE and ACT
  - measured (cost-model timeline): 196.6us vs 215.7us baseline, ACT busy
    149.5us (76%), PE busy 161us (82%), rel err 6.3e-3 vs the 2e-2 gate
"""

import numpy as np

T = 2048
DMODEL = 2048
NKV = 8
GROUP = 4
DH = 64
SINK = 64
NST = 17  # s-tiles: 16 full 128-tiles of new tokens + 1 sink tile (64 rows)
SCALE = 0.125  # 1/sqrt(64)

_CACHE = {}


def _build_nc():
    import concourse.bass as bass
    import concourse.mybir as mybir
    import concourse.tile as tile
    from concourse import bacc

    f32 = mybir.dt.float32
    bf16 = mybir.dt.bfloat16

    nc = bacc.Bacc("TRN2", target_bir_lowering=False, debug=False, num_devices=NKV)

    xT = nc.declare_dram_parameter("xT", [DMODEL, T], bf16, isOutput=False).ap()
    wq01 = nc.declare_dram_parameter("wq01", [DMODEL, 2 * DH], bf16, isOutput=False).ap()
    wq23 = nc.declare_dram_parameter("wq23", [DMODEL, 2 * DH], bf16, isOutput=False).ap()
    wkv = nc.declare_dram_parameter("wkv", [DMODEL, 2 * DH], bf16, isOutput=False).ap()
    wo = nc.declare_dram_parameter("wo", [GROUP * DH, DMODEL], bf16, isOutput=False).ap()
    sink_kT = nc.declare_dram_parameter("sink_kT", [DH, SINK], bf16, isOutput=False).ap()
    sink_v = nc.declare_dram_parameter("sink_v", [128, DH], bf16, isOutput=False).ap()
    # cos and sin rope tables packed into one tensor: one DMA instead of
    # two (HWDGE descriptor generation is a serial global resource)
    cs_tbl = nc.declare_dram_parameter("cs_tbl", [128, 2 * T], bf16, isOutput=False).ap()
    # swap permutation: swp[p, i] = 1 iff p = (i+32 mod 64) within i's 64-block
    swp = nc.declare_dram_parameter("swp", [128, 128], bf16, isOutput=False).ap()
    # bf16 identity for PE transposes (1 cycle/row vs 2 for f32)
    id128 = nc.declare_dram_parameter("id128", [128, 128], bf16, isOutput=False).ap()
    out = nc.declare_dram_parameter("out", [T, DMODEL], bf16, isOutput=True).ap()

    with (
        tile.TileContext(nc) as tc,
        tc.tile_pool(name="persist", bufs=1) as persist,
        # scores psum (4 banks double buffered): the exp stream must never
        # wait on projection psum
        tc.tile_pool(name="psA", bufs=2, space="PSUM") as psA,
        tc.tile_pool(name="pm", bufs=1) as pm,
    ):
        # q01/q23: heads (0,1) and (2,3) on partition halves; odd q heads are
        # extracted to base-partition-0 tiles qx1/qx3 post-rope.
        q01 = persist.tile([128, T], bf16, tag="q01")
        q23 = persist.tile([128, T], bf16, tag="q23")
        qx1 = persist.tile([DH, T], bf16, tag="qx1")
        qx3 = persist.tile([DH, T], bf16, tag="qx3")
        kk = persist.tile([DH, T], bf16, tag="kk")
        vTt = persist.tile([DH, T], bf16, tag="vTt")
        v_sb = persist.tile([128, NST, DH + 1], bf16, tag="v_sb")
        ctxT = [persist.tile([128, T], bf16, tag=f"ctxT{j}", name=f"ctxT{j}") for j in range(2)]
        # normalized ctx in [token, head-dim] layout: O_sb[:, ti, g*64:(g+1)*64]
        O_sb = persist.tile([128, 16, GROUP * DH], bf16, tag="O_sb")
        sink_kT_sb = persist.tile([DH, SINK], bf16, tag="sink_kT")
        # merged sink weights: one tile per block PAIR — head A's sink exp
        # rows live at partitions 0:64, head B's at 64:128 (one exp instead
        # of two; the base-64 ctx matmul reads the duplicated sink v rows)
        wTs = [
            persist.tile([128, 1024], bf16, tag=f"wTs{p}", name=f"wTs{p}")
            for p in range(4)
        ]
        # block (0,3)'s first 4 s-tiles: a small dedicated tile lets head 3
        # join the early exp stream without a wT rotation conflict (its
        # rotating buffer is still owned by block (0,0) during phase 1)
        wT03e = persist.tile([128, 4, 1024], bf16, tag="wT03e")
        swp_sb = persist.tile([128, 128], bf16, tag="swp")
        id128_sb = persist.tile([128, 128], bf16, tag="id128_sb")

        xTr = xT.rearrange("(k p) t -> p k t", p=128)
        wkvr = wkv.rearrange("(k p) m -> p k m", p=128)
        wq01r = wq01.rearrange("(k p) m -> p k m", p=128)
        wq23r = wq23.rearrange("(k p) m -> p k m", p=128)

        # wT buffers rotate out of pm, triple buffered (sink weights live
        # in the per-pair wTs tiles instead)
        def new_wT():
            return pm.tile([128, 16, 1024], bf16, tag="wT", bufs=3, name="wTm")

        def emit_tile(qt, wT, c0, st):
            """One s-tile of scores (PE) + its exp (ACT).

            The PE queue is in-order and scores psum is double buffered, so
            bursts of more than ~2 of these couple the PE to the ACT exp
            pace; callers must interleave them with other PE work.
            """
            sps = psA.tile([128, 1024], f32, tag="sps", name="sps")
            lhsT = kk[:, st * 128 : (st + 1) * 128]
            for u in range(2):
                nc.tensor.matmul(
                    sps[:, u * 512 : (u + 1) * 512],
                    lhsT=lhsT,
                    rhs=qt[:, c0 + u * 512 : c0 + (u + 1) * 512],
                    start=True,
                    stop=True,
                )
            nc.scalar.activation(
                out=wT[:, st, :],
                in_=sps,
                func=mybir.ActivationFunctionType.Exp,
                scale=SCALE,
            )

        def emit_half(qt, wT, st, u, c0=0):
            """512-wide emission: half the block width per exp.  Used to
            start the ACT exp stream as soon as chunk 0 is roped, and to
            split the last block so its ctx/output chain overlaps the final
            exps (costs one extra exp's fixed overhead per tile)."""
            sps = psA.tile([128, 512], f32, tag="sps", name="sps")
            lhsT = kk[:, st * 128 : (st + 1) * 128]
            nc.tensor.matmul(
                sps,
                lhsT=lhsT,
                rhs=qt[:, c0 + u * 512 : c0 + (u + 1) * 512],
                start=True,
                stop=True,
            )
            nc.scalar.activation(
                out=wT[:, st, u * 512 : (u + 1) * 512],
                in_=sps,
                func=mybir.ActivationFunctionType.Exp,
                scale=SCALE,
            )

        def emit_sink(pair, qA, qB, c0=0, u=None):
            """Merged sink scores for a block pair: head A's rows at
            partitions 0:64, head B's at 64:128, one exp for both."""
            if u is None:
                w, sl = 1024, slice(0, 1024)
            else:
                w, sl = 512, slice(u * 512, (u + 1) * 512)
            sps = psA.tile([128, w], f32, tag="sps", name="sps")
            for hb, qt in ((0, qA), (1, qB)):
                for uu in range(w // 512):
                    t0 = c0 + (sl.start if u is not None else uu * 512)
                    nc.tensor.matmul(
                        sps[hb * 64 : hb * 64 + 64, uu * 512 : uu * 512 + 512],
                        lhsT=sink_kT_sb,
                        rhs=qt[:, t0 : t0 + 512],
                        start=True,
                        stop=True,
                    )
            nc.scalar.activation(
                out=wTs[pair][:, sl],
                in_=sps,
                func=mybir.ActivationFunctionType.Exp,
                scale=SCALE,
            )

        def transpose_v(st):
            # v^T -> v_sb through a scores-psum slot (bf16 transpose via the
            # bf16 identity: 1 cyc/row); no extra PSUM banks
            sps = psA.tile([128, 1024], bf16, tag="sps", name="sps")
            nc.tensor.transpose(
                sps[:, 0:DH], vTt[:, st * 128 : (st + 1) * 128],
                id128_sb[0:DH, 0:DH],
            )
            nc.vector.tensor_copy(out=v_sb[:, st, 0:DH], in_=sps[:, 0:DH])

        pend = []  # queued emission thunks, paced via pump()

        def pump(n):
            for _ in range(min(n, len(pend))):
                pend.pop(0)()

        with (
            tc.tile_pool(name="psB", bufs=1, space="PSUM") as psB,
            tc.tile_pool(name="pw", bufs=1) as pw,
            tc.tile_pool(name="px", bufs=2) as px,
        ):
            # ---- phase 1: projections + per-chunk rope ----
            # first weight piece alone so the k=0 matmuls start at ~1.5us
            # lo/hi halves as separate tiles: reader-after-writer ordering
            # is per tile, so k<8 matmuls issued after the hi-half DMAs must
            # not inherit a dependency on them
            wq01_lo = pw.tile([128, 8, 2 * DH], bf16, tag="wq01_lo")
            wq01_hi = pw.tile([128, 8, 2 * DH], bf16, tag="wq01_hi")
            wq23_lo = pw.tile([128, 8, 2 * DH], bf16, tag="wq23_lo")
            wq23_hi = pw.tile([128, 8, 2 * DH], bf16, tag="wq23_hi")
            wkv_lo = pw.tile([128, 8, 2 * DH], bf16, tag="wkv_lo")
            wkv_hi = pw.tile([128, 8, 2 * DH], bf16, tag="wkv_hi")

            def wq01s(k):
                return wq01_lo[:, k] if k < 8 else wq01_hi[:, k - 8]

            def wq23s(k):
                return wq23_lo[:, k] if k < 8 else wq23_hi[:, k - 8]

            def wkvs(k):
                return wkv_lo[:, k] if k < 8 else wkv_hi[:, k - 8]


            sinkv_st = pw.tile([128, DH], bf16, tag="sinkv_st")

            def rope_chunk(tgt, cs, pp, cs_c):
                """tgt[0:pp, cs] <- tgt*C + swap32(tgt)*S on token slice cs;
                cs_c holds this chunk's 512 cos cols then 512 sin cols."""
                n = cs.stop - cs.start
                aux = psB.tile([128, 512], f32, tag="aux", name="aux")
                nc.tensor.matmul(
                    aux[0:pp, 0:n],
                    lhsT=swp_sb[0:pp, 0:pp],
                    rhs=tgt[0:pp, cs],
                    start=True,
                    stop=True,
                )
                swt = px.tile([128, 512], bf16, tag="sw")
                nc.vector.tensor_mul(swt[0:pp, 0:n], aux[0:pp, 0:n], cs_c[0:pp, 512 : 512 + n])
                nc.vector.tensor_mul(tgt[0:pp, cs], tgt[0:pp, cs], cs_c[0:pp, 0:n])
                nc.vector.tensor_add(tgt[0:pp, cs], tgt[0:pp, cs], swt[0:pp, 0:n])

            wT_early = {}
            qsrc = [q01[0:DH, :], qx1, q23[0:DH, :], qx3]

            # PE p-state ramp: matmuls run 2-4x slow until ~3us of
            # continuous PE activity.  Warm the ramp on a zero tile while
            # the first x/weight DMAs are still in flight, so the real
            # projection stream starts at full speed.
            warm = px.tile([128, 128], bf16, tag="warm", bufs=1)
            nc.vector.memset(warm, 0.0)
            wps = psB.tile([128, 512], f32, tag="aux", name="wps")
            for _ in range(4):
                nc.tensor.matmul(
                    wps[:, 0:128], lhsT=warm, rhs=warm, start=True, stop=True
                )

            for c in range(4):  # token chunks of 512
                cs_c = pw.tile([128, 1024], bf16, tag="cs", bufs=2, name="cs_c")
                q01ps = psB.tile([128, 512], f32, tag="q01ps")
                q23ps = psB.tile([128, 512], f32, tag="q23ps")
                kvps = psB.tile([128, 512], f32, tag="kvps")
                cs = slice(c * 512, (c + 1) * 512)
                # kv + q01 matmuls stream per x quad; q23's are deferred past
                # the kk/q01 ropes (they gate the early exp stream), so the
                # first exp fires after ~70 PE matmuls instead of ~105
                xts = []
                for quad in range(4):
                    # DMA queues: transfers serialize per HWDGE queue, so the
                    # startup x stream alternates the SP and ACT queues (ACT
                    # is idle until its first exp ~15us) and every constant
                    # rides the DVE queue — chunk 0 then lands by ~5us
                    # instead of ~14us and the exp stream starts ~8us sooner.
                    if c == 0 and quad == 0:
                        # first 2 x pieces + first weight halves alone so the
                        # first projection matmul starts at ~2.5us.  The DMA
                        # transfer stage and HWDGE are single global
                        # resources, so issue ORDER is everything: hi weights
                        # after quad 1 (needed at k=8), tables after quad 2
                        # (needed by the ropes).
                        xa = px.tile([128, 1, 512], bf16, tag="xt0", bufs=1)
                        nc.sync.dma_start(out=xa, in_=xTr[:, 0:1, cs])
                        nc.sync.dma_start(out=wkv_lo[:, 0:2], in_=wkvr[:, 0:2])
                        nc.sync.dma_start(out=wq01_lo[:, 0:2], in_=wq01r[:, 0:2])
                        xb = px.tile([128, 3, 512], bf16, tag="xt1", bufs=1)
                        nc.sync.dma_start(out=xb, in_=xTr[:, 1:4, cs])
                        nc.sync.dma_start(out=wkv_lo[:, 2:8], in_=wkvr[:, 2:8])
                        nc.sync.dma_start(out=wq01_lo[:, 2:8], in_=wq01r[:, 2:8])
                        parts = [(xa, 0, 1), (xb, 1, 3)]
                    else:
                        xt = px.tile([128, 4, 512], bf16, tag="xt", bufs=7)
                        nc.sync.dma_start(
                            out=xt,
                            in_=xTr[:, quad * 4 : (quad + 1) * 4, cs],
                        )
                        parts = [(xt, quad * 4, 4)]
                        if c == 0 and quad == 1:
                            # hi weight halves: consumed by this chunk's k=8
                            nc.sync.dma_start(out=wkv_hi, in_=wkvr[:, 8:16])
                            nc.sync.dma_start(out=wq01_hi, in_=wq01r[:, 8:16])
                        elif c == 0 and quad == 3:
                            # rope tables + sink keys AFTER the last x quad,
                            # and only chunk 0's 512-col cos/sin slices: the
                            # ropes are the exp-stream gate and only ever
                            # need the current chunk's columns
                            nc.sync.dma_start(
                                out=cs_c[:, 0:512], in_=cs_tbl[:, 0:512]
                            )
                            nc.sync.dma_start(
                                out=cs_c[:, 512:1024], in_=cs_tbl[:, T : T + 512]
                            )
                            nc.sync.dma_start(out=swp_sb, in_=swp)
                            nc.sync.dma_start(out=sink_kT_sb, in_=sink_kT)
                            # q23 weights leave the critical front: their
                            # matmuls only run after the early emits
                            nc.sync.dma_start(out=wq23_lo, in_=wq23r[:, 0:8])
                            nc.sync.dma_start(out=wq23_hi, in_=wq23r[:, 8:16])
                        elif c >= 1 and quad == 0:
                            # this chunk's cos/sin slices: land well before
                            # its ropes
                            nc.sync.dma_start(
                                out=cs_c[:, 0:512],
                                in_=cs_tbl[:, c * 512 : (c + 1) * 512],
                            )
                            nc.sync.dma_start(
                                out=cs_c[:, 512:1024],
                                in_=cs_tbl[:, T + c * 512 : T + (c + 1) * 512],
                            )
                        if c == 1 and quad == 0:
                            # consumed by the v transposes / phase 2; SWDGE
                            # keeps the HWDGE queue free
                            nc.gpsimd.dma_start(out=sinkv_st, in_=sink_v)
                            nc.gpsimd.dma_start(out=id128_sb, in_=id128)
                    xts.extend(parts)
                    for xtile, k0, kn in parts:
                        for k4 in range(kn):
                            k = k0 + k4
                            nc.tensor.matmul(
                                kvps, lhsT=wkvs(k),
                                rhs=xtile[:, k4, :], start=(k == 0), stop=(k == 15),
                            )
                            nc.tensor.matmul(
                                q01ps, lhsT=wq01s(k), rhs=xtile[:, k4, :],
                                start=(k == 0), stop=(k == 15),
                            )
                        pump(2)
                if c == 1:
                    # q01 first: the u1-half emits of st0..3 need only q01's
                    # rope (keys are chunk 0's), so they fire ~2us sooner
                    nc.vector.tensor_copy(out=q01[:, cs], in_=q01ps)
                else:
                    nc.vector.tensor_copy(out=kk[:, cs], in_=kvps[0:DH, :])
                    nc.vector.tensor_copy(out=q01[:, cs], in_=q01ps)
                nc.vector.tensor_copy(out=vTt[:, cs], in_=kvps[DH:128, :])
                # flipped ctx consumes every s-tile per token tile, so v_sb
                # must be complete before the first ctx block: queue the
                # chunk's v transposes as pend work
                if c == 1:
                    for st in range(0, 8):
                        pend.append(lambda st=st: transpose_v(st))
                elif c > 1:
                    for st in range(4 * c, 4 * c + 4):
                        pend.append(lambda st=st: transpose_v(st))
                if c > 0:
                    pump(2)  # queued scores read already-roped chunks: free
                    # PE work while the kk/q01 copies land
                # rope (k first: it gates all heads' scores); pumped tiles
                # read already-roped chunks, covering the copy/swap latency
                if c == 1:
                    rope_chunk(q01, cs, 128, cs_c)
                    nc.gpsimd.dma_start(out=qx1[:, cs], in_=q01[DH:128, cs])
                    pend.extend(
                        (lambda st=st: emit_half(qsrc[0], wT_early[0], st, 1))
                        for st in range(0, 4)
                    )
                    pend.append(lambda: emit_sink(0, qsrc[0], qsrc[1], u=1))
                    pend.extend(
                        (lambda st=st: emit_half(qsrc[1], wT_early[1], st, 1))
                        for st in range(0, 4)
                    )
                    pump(5)
                    nc.vector.tensor_copy(out=kk[:, cs], in_=kvps[0:DH, :])
                    rope_chunk(kk, cs, DH, cs_c)
                    pump(2)
                else:
                    rope_chunk(kk, cs, DH, cs_c)
                    pump(1)
                    rope_chunk(q01, cs, 128, cs_c)
                    pump(1)
                # odd head to a base-partition-0 tile (post-rope, per chunk);
                # SWDGE: keeps the contended HWDGE free for x/weight loads
                if c != 1:
                    nc.gpsimd.dma_start(out=qx1[:, cs], in_=q01[DH:128, cs])
                # early exp: queue scores+exp for the chunk-0 blocks of heads
                # 0/1 as soon as the needed k/q chunks are roped; pump() paces
                # them between projection quads so the ACT engine (the ~141us
                # serial backbone) starts early without ever blocking the
                # in-order PE queue on the exp stream
                if c == 0:
                    # earliest possible exp stream: 512-wide score halves for
                    # the sink + chunk-0 keys x chunk-0 queries of heads 0/1,
                    # pumped by the remaining q23 parts below
                    wT_early[0] = new_wT()
                    wT_early[1] = new_wT()
                    pend.extend(
                        (lambda st=st: emit_half(qsrc[0], wT_early[0], st, 0))
                        for st in range(0, 4)
                    )
                    pend.append(lambda: emit_sink(0, qsrc[0], qsrc[1], u=0))
                    pend.extend(
                        (lambda st=st: emit_half(qsrc[1], wT_early[1], st, 0))
                        for st in range(0, 4)
                    )
                    # fire the early emits BEFORE the q23 matmuls: q23 waits
                    # on its late-loaded weights, and the PE has nothing else
                    # this early — a paced cram here is free
                    pump(5)
                    for xtile, k0, kn in xts:
                        for k4 in range(kn):
                            k = k0 + k4
                            nc.tensor.matmul(
                                q23ps, lhsT=wq23s(k),
                                rhs=xtile[:, k4, :],
                                start=(k == 0), stop=(k == 15),
                            )
                        pump(2)
                elif c == 1:
                    for g in range(2):
                        pend.extend(
                            (lambda g=g, st=st: emit_tile(qsrc[g], wT_early[g], 0, st))
                            for st in range(4, 8)
                        )
                elif c >= 2:
                    # g=2 (block (0,2), wT02) joins here: its q23 chunks 0/1
                    # were roped in earlier chunks, so full-width is fine
                    for g in range(3):
                        pend.extend(
                            (lambda g=g, st=st: emit_tile(qsrc[g], wT_early[g], 0, st))
                            for st in range(4 * c, 4 * c + 4)
                        )
                # deferred q23 matmuls + its rope
                if c > 0:
                    for xtile, k0, kn in xts:
                        for k4 in range(kn):
                            k = k0 + k4
                            nc.tensor.matmul(
                                q23ps, lhsT=wq23s(k), rhs=xtile[:, k4, :],
                                start=(k == 0), stop=(k == 15),
                            )
                        pump(3)
                nc.vector.tensor_copy(out=q23[:, cs], in_=q23ps)
                rope_chunk(q23, cs, 128, cs_c)
                nc.gpsimd.dma_start(out=qx3[:, cs], in_=q23[DH:128, cs])
                # block (0,2)'s early tiles queue only after q23's rope: its
                # scores read the q23 chunk roped just above
                if c == 0:
                    wT_early[2] = new_wT()
                    pend.append(lambda: emit_sink(1, qsrc[2], qsrc[3], u=0))
                    for st in range(0, 4):
                        pend.append(
                            lambda st=st: emit_half(qsrc[2], wT_early[2], st, 0)
                        )
                        pend.append(
                            lambda st=st: emit_half(qsrc[3], wT03e, st, 0)
                        )
                elif c == 1:
                    pend.append(lambda: emit_sink(1, qsrc[2], qsrc[3], u=1))
                    for st in range(0, 4):
                        pend.append(
                            lambda st=st: emit_half(qsrc[2], wT_early[2], st, 1)
                        )
                        pend.append(
                            lambda st=st: emit_half(qsrc[3], wT03e, st, 1)
                        )
                    pend.extend(
                        (lambda st=st: emit_tile(qsrc[2], wT_early[2], 0, st))
                        for st in range(4, 8)
                    )
                if c == 3:
                    # c=1 block pairs' merged sinks: all q is roped now
                    pend.append(lambda: emit_sink(2, qsrc[0], qsrc[1], c0=1024))
                    pend.append(lambda: emit_sink(3, qsrc[2], qsrc[3], c0=1024))
            pump(len(pend))  # drain any leftovers (ACT is behind PE here)
            # phase-2-only constants, placed after the chunk loop so their
            # consumers never head-block the in-order DVE/Pool queues while
            # the projection copies are pending
            nc.vector.tensor_copy(out=v_sb[:, NST - 1, 0:DH], in_=sinkv_st)
            nc.vector.memset(v_sb[:, :, DH : DH + 1], 1.0)

        # ---- phase 2+3: flipped ctx + transposes, then per-512 output ----
        with (
            tc.tile_pool(name="psOC", bufs=2, space="PSUM") as psOC,
            tc.tile_pool(name="psD", bufs=2, space="PSUM") as psD,
            tc.tile_pool(name="pLate", bufs=1) as pLate,
        ):
            wo_sb = pLate.tile([128, 2, DMODEL], bf16, tag="wo_sb")
            nc.sync.dma_start(out=wo_sb, in_=wo.rearrange("(a p) n -> p a n", p=128))

            # yps and transpose psums share one tag (bank-granular slots):
            # psA's 4 banks + psOC's 2 + these 2 fill PSUM exactly
            def scratch_ps():
                return psD.tile([128, 512], f32, tag="ps", name="ps")

            # --- rate-paced emission: the in-order PE queue means a queued
            # emit blocks ALL later PE work until its scores psum frees (the
            # ACT exp two back).  Feeds must therefore be spread at no more
            # than ~1 per exp-duration of interleaved PE work, or real work
            # gets pushed past the end of the exp stream.  slot(ns) is called
            # at every interleave point with the PE-ns just appended and pops
            # pending emits at their own pace.
            pend2 = []  # (thunk, pace_ns, group)
            pacc = [0.0]

            def slot(ns):
                pacc[0] += ns
                pops = 0
                while pend2 and pacc[0] >= pend2[0][1] and pops < 2:
                    _, pace, _ = pend2[0]
                    pacc[0] -= pace
                    pend2.pop(0)[0]()
                    pops += 1

            def queue_block(thunks, pace, group=""):
                if not pend2:
                    pacc[0] = min(pacc[0], 1000.0)
                pend2.extend((t, pace, group) for t in thunks)

            def drain_upto(group):
                """Force-run queued emits from the front until none of the
                given group remain (safety net under the slot() pacing —
                every ctx must have its block's exps fully issued)."""
                while any(g == group for _, _, g in pend2):
                    pend2.pop(0)[0]()

            def full_tiles(c, g, wT):
                return [
                    (lambda st=st: emit_tile(qsrc[g], wT, c * 1024, st))
                    for st in range(16)
                ]

            def ctx_block(c, g, wT, tts=range(8), wt_e4=None):
                """Flipped ctx for one block: per 128-token tile, the merged
                sink tile first (head parity picks the partition half), then
                the 16 key tiles, all with wT as the stationary operand;
                normalize into O_sb via the denominator column."""
                hb = g % 2
                pair = (c << 1) | (g >> 1)
                for tt in tts:
                    oc = psOC.tile([128, DH + 1], f32, tag="oc", name="oc")
                    cols = slice(tt * 128, (tt + 1) * 128)
                    slot(232)
                    nc.tensor.matmul(
                        oc,
                        lhsT=wTs[pair][hb * 64 : hb * 64 + 64, cols],
                        rhs=v_sb[hb * 64 : hb * 64 + 64, NST - 1, :],
                        start=True,
                        stop=False,
                    )
                    for i, st in enumerate(range(16)):
                        if i == 8:
                            slot(232)
                        src_wT = wt_e4 if (wt_e4 is not None and st < 4) else wT
                        nc.tensor.matmul(
                            oc,
                            lhsT=src_wT[:, st, cols],
                            rhs=v_sb[:, st, :],
                            start=False,
                            stop=(i == 15),
                        )
                    rec = pm.tile([128, 1], f32, tag="rec", bufs=2, name="rec")
                    nc.vector.reciprocal(rec, oc[:, DH : DH + 1])
                    nc.vector.tensor_scalar_mul(
                        O_sb[:, c * 8 + tt, g * DH : (g + 1) * DH],
                        oc[:, 0:DH], rec,
                    )

            def tp_burst(c, j, tts=range(8)):
                """O_sb[t, headpair j] -> ctxT[j][d, t] for the tiles of
                token half c (PE transpose + DVE drain per tile)."""
                for tt in tts:
                    slot(181)
                    tp = psD.tile([128, 128], bf16, tag="ps", name="tp")
                    nc.tensor.transpose(
                        tp, O_sb[:, c * 8 + tt, j * 128 : (j + 1) * 128], id128_sb
                    )
                    ti = c * 8 + tt
                    nc.vector.tensor_copy(
                        out=ctxT[j][:, ti * 128 : (ti + 1) * 128], in_=tp
                    )

            def ny_block(c, u, mode):
                """Output projection for 512 tokens.  mode: "steady" (DVE
                copies, psD yps), "mid" (exp stream still running: DVE-only
                copies, yps alternating psD/psOC — psOC's ctx accumulators
                are done by then — to break the two-buffer ping-pong without
                touching psA or the ACT queue), "tail" (exps over: copies
                alternate DVE/ACT, yps alternate psD/psA)."""
                t0 = c * 1024 + u * 512
                for tt4 in range(4):  # output projection per 128 tokens
                    tt = t0 // 128 + tt4
                    y_sb = pLate.tile([128, DMODEL], bf16, tag="y_sb", bufs=3, name="y_sb")
                    for nck in range(4):
                        slot(427)
                        if mode == "tail" and nck % 2 == 1:
                            yps = psA.tile([128, 512], f32, tag="sps", name="sps")
                        elif mode == "mid" and nck % 2 == 1:
                            yps = psOC.tile([128, 512], f32, tag="oc", name="oc")
                        else:
                            yps = scratch_ps()
                        for j in range(2):
                            nc.tensor.matmul(
                                yps,
                                lhsT=ctxT[j][:, tt * 128 : (tt + 1) * 128],
                                rhs=wo_sb[:, j, nck * 512 : (nck + 1) * 512],
                                start=(j == 0),
                                stop=(j == 1),
                            )
                        ysl = slice(nck * 512, (nck + 1) * 512)
                        if mode == "tail" and nck % 2 == 1:
                            # ACT is idle once its exp stream ends; share the
                            # tail copies between DVE and ACT
                            nc.scalar.copy(out=y_sb[:, ysl], in_=yps)
                        else:
                            nc.vector.tensor_copy(out=y_sb[:, ysl], in_=yps)
                        if mode == "tail" and tt4 == 3 and nck == 1:
                            # very last tile: DMA the first half early so the
                            # final transfer overlaps the remaining copies
                            nc.sync.dma_start(
                                out=out[tt * 128 : (tt + 1) * 128, 0:1024],
                                in_=y_sb[:, 0:1024],
                            )
                    if mode == "tail" and tt4 == 3:
                        nc.sync.dma_start(
                            out=out[tt * 128 : (tt + 1) * 128, 1024:2048],
                            in_=y_sb[:, 1024:2048],
                        )
                    else:
                        nc.sync.dma_start(out=out[tt * 128 : (tt + 1) * 128, :], in_=y_sb)

            # software pipeline.  wT rotation (bufs=3, allocations e0,e1,e2
            # in phase 1 then w03,w10,w11,w12,w13): each block's emission can
            # only be queued after the ctx that reads the buffer it reuses.
            # The last block (1,3) is emitted as 512-wide halves so its
            # ctx/transpose/output chain for the first half overlaps the
            # second half's exps.
            ctx_block(0, 0, wT_early[0])
            wT03 = new_wT()
            queue_block(full_tiles(0, 3, wT03)[4:], 1000, "03")
            ctx_block(0, 1, wT_early[1])
            wT10 = new_wT()
            queue_block(full_tiles(1, 0, wT10), 1000, "10")
            tp_burst(0, 0)
            ctx_block(0, 2, wT_early[2])
            wT11 = new_wT()
            queue_block(full_tiles(1, 1, wT11), 1000, "11")
            drain_upto("03")
            ctx_block(0, 3, wT03, wt_e4=wT03e)
            wT12 = new_wT()
            queue_block(full_tiles(1, 2, wT12), 1000, "12")
            tp_burst(0, 1)
            ny_block(0, 0, "steady")
            drain_upto("10")
            ctx_block(1, 0, wT10)
            wT13 = new_wT()
            queue_block(
                [
                    (lambda st=st: emit_half(qsrc[3], wT13, st, 0, c0=1024))
                    for st in range(16)
                ],
                620, "a",
            )
            queue_block(
                [
                    (lambda st=st: emit_half(qsrc[3], wT13, st, 1, c0=1024))
                    for st in range(16)
                ],
                620, "b",
            )
            ny_block(0, 1, "steady")
            drain_upto("11")
            ctx_block(1, 1, wT11)
            tp_burst(1, 0)
            drain_upto("12")
            ctx_block(1, 2, wT12)
            drain_upto("a")
            ctx_block(1, 3, wT13, tts=range(0, 4))
            tp_burst(1, 1, tts=range(0, 4))
            ny_block(1, 0, "mid")
            drain_upto("b")
            ctx_block(1, 3, wT13, tts=range(4, 8))
            tp_burst(1, 1, tts=range(4, 8))
            ny_block(1, 1, "tail")

    nc.compile()
    return nc


def _host_inputs(x, kv_cache, Wq, Wk, Wv, Wo, start_pos):
    """Build the 8 per-core input dicts."""
    from ml_dtypes import bfloat16

    f32 = np.float32
    xT = np.ascontiguousarray(np.asarray(x, f32)[0].T.astype(bfloat16))

    inv_freq = (1.0 / (10000.0 ** (np.arange(0, DH, 2, dtype=f32) / DH))).astype(f32)
    pos = np.arange(start_pos, start_pos + T, dtype=f32)
    ang = pos[:, None] * inv_freq[None, :]
    cosT = np.cos(ang).T.astype(f32)  # (32, T)
    sinT = np.sin(ang).T.astype(f32)
    cosb = np.concatenate([cosT] * 4, axis=0)
    sinb = np.concatenate([-sinT, sinT, -sinT, sinT], axis=0)
    cs_tbl = np.ascontiguousarray(np.concatenate([cosb, sinb], axis=1)).astype(bfloat16)

    # 32-row swap within each 64-block: swp[p, i] = 1 iff p = swap(i)
    swp = np.zeros((128, 128), dtype=bfloat16)
    for i in range(128):
        blk = (i // 64) * 64
        swp[blk + ((i - blk) + 32) % 64, i] = 1
    id128 = np.eye(128, dtype=bfloat16)

    Wq = np.asarray(Wq, f32)
    Wk = np.asarray(Wk, f32)
    Wv = np.asarray(Wv, f32)
    Wo = np.asarray(Wo, f32)
    kv_cache = np.asarray(kv_cache, f32)

    in_maps = []
    for i in range(NKV):
        sink = kv_cache[0, i, :SINK, :]
        sink_kT = np.ascontiguousarray(sink.T).astype(bfloat16)
        in_maps.append(
            {
                "xT": xT,
                "wq01": np.ascontiguousarray(
                    Wq[:, i * GROUP * DH : i * GROUP * DH + 2 * DH]
                ).astype(bfloat16),
                "wq23": np.ascontiguousarray(
                    Wq[:, i * GROUP * DH + 2 * DH : (i + 1) * GROUP * DH]
                ).astype(bfloat16),
                "wkv": np.ascontiguousarray(
                    np.concatenate(
                        [Wk[:, i * DH : (i + 1) * DH], Wv[:, i * DH : (i + 1) * DH]],
                        axis=1,
                    )
                ).astype(bfloat16),
                "wo": np.ascontiguousarray(
                    Wo[i * GROUP * DH : (i + 1) * GROUP * DH, :]
                ).astype(bfloat16),
                "sink_kT": sink_kT,
                "sink_v": np.ascontiguousarray(
                    np.concatenate([sink, sink], axis=0)
                ).astype(bfloat16),
                "cs_tbl": cs_tbl,
                "swp": swp,
                "id128": id128,
            }
        )
    return in_maps


def run(inputs, trace=False, trace_kwargs=None):
    """Run the 8-core kernel; returns (y, BassKernelResults)."""
    from concourse.bass_utils import run_bass_kernel_spmd

    if "nc" not in _CACHE:
        _CACHE["nc"] = _build_nc()
    nc = _CACHE["nc"]

    start_pos = int(np.asarray(inputs["start_pos"]))
    in_maps = _host_inputs(
        inputs["x"], inputs["kv_cache"], inputs["Wq"], inputs["Wk"], inputs["Wv"],
        inputs["Wo"], start_pos,
    )
    kwargs = {}
    if trace:
        kwargs["trace"] = True
        if trace_kwargs:
            kwargs["trace_kwargs"] = trace_kwargs
    res = run_bass_kernel_spmd(nc, in_maps, core_ids=list(range(NKV)), **kwargs)

    y = res.results[0]["out"].astype(np.float64)
    for i in range(1, NKV):
        y += res.results[i]["out"]
    y = (y + np.asarray(inputs["bo"], np.float64)[None, :]).astype(np.float32)
    return y[None], res


def kernel(**inputs):
    y, _ = run(inputs)
    return y
